# revision 19
# baseline (speedup 1.0000x reference)
"""Conditional (class-routed) 3x3 SAME conv, data-parallel over batch on 8 TRN2 cores.

Strategy (v2 - fp8 DoubleRow residual scheme):
  - Flat conv: zero-padded x [CIN, 66, 66] viewed as a flat [CIN, 4356] plane
    makes the 2D conv a 1D conv in the flat index (the 2 pad columns between
    rows absorb row wrap), so every tap window is a contiguous slice.  We
    compute 4222 flat outputs per (sample, f-half) (64 rows x 66, garbage at
    the 2 pad columns per row, stripped on host).
  - fp8e4 (e4m3) matmuls in MatmulPerfMode.DoubleRow: lhsT [K,2,M] x
    rhs [K,2,N] -> out[M,N] contracts a PAIR of products per output at half
    the per-row cost of fp16 -> 4x cheaper per product-term.
  - Residual precision scheme (kernel pre-scaled by 64, split k = k_hi+k_lo
    and x = x_hi+x_lo in e4m3):
      MM_A(tap t in 0..8): slots (x_hi*k_hi[t], x_lo*k_hi[t]) - exact-x
      MM_C(p in 0..2):     slots (x_hi*k_lo[t=p], x_hi*k_lo[t=p+3]) -
        k_lo corrections for the kh=0 and kh=1 tap rows, packed 2 per matmul
        using an extra "x_hi shifted by 66" SBUF plane (window offsets of the
        paired taps differ by exactly one padded row).
    Dropped terms: x*k_lo on the kh=2 tap row (dominant error, ~1.6e-2 rel)
    and x_lo*k_lo (~7e-4).  12 DoubleRow matmuls per chunk vs 9 fp16
    matmuls: 0.78x the fp16 PE-roofline time.
  - PSUM eviction applies out = psum/64 + bias, alternating DVE/ACT engines.
  - Host: gather per-sample expert kernels, build hi/lo e4m3 splits + the
    shifted plane, strip flat-index garbage and concatenate shards.
"""

import numpy as np

_B, _H, _W, _CIN = 32, 64, 64, 128
_F, _KH, _KW = 256, 3, 3
_NCORES = 8
_BPC = _B // _NCORES          # 4 samples per core
_HP, _WP = _H + 2, _W + 2     # 66, 66 (zero-padded)
_FLAT = _HP * _WP             # 4356 flat padded positions
_NOUT = (_H - 1) * _WP + _W   # 4222 flat outputs (incl. per-row garbage cols)
_FH = 128                     # output-channel half (PSUM partition dim)
_NFH = _F // _FH              # 2
_NTAP = _KH * _KW             # 9
_KSCALE = 64.0                # kernel pre-scale (keeps e4m3 out of subnormals)

_OFFS = [kh * _WP + kw for kh in range(_KH) for kw in range(_KW)]
_NPAIR = 3                    # MM_C count: pair p corrects taps (p, p+3)
_NMM = _NTAP + _NPAIR         # 12 matmuls per chunk
# small chunk FIRST: it needs only a sliver of x, so the PE starts early and
# the expensive 512-chunks run fully pstate-ramped
_CHUNKS = [(0, _NOUT - 8 * 512)] + [(126 + i * 512, 512) for i in range(8)]
_XSPLITS = (0, 272, 832, 1332, 2356, 3380, _FLAT)   # x DMA piece boundaries

_nc = None
_E4M3 = None


def _e4m3():
    global _E4M3
    if _E4M3 is None:
        import concourse.mybir as mybir
        _E4M3 = np.dtype(mybir.dt.np(mybir.dt.float8e4))
    return _E4M3


def _build_nc():
    import concourse.bacc as bacc
    import concourse.mybir as mybir
    import concourse.tile as tile
    from concourse.tile_rust import add_dep_helper

    f32 = mybir.dt.float32
    f16 = mybir.dt.float16
    f8 = mybir.dt.float8e4
    DR = mybir.MatmulPerfMode.DoubleRow
    ident = mybir.ActivationFunctionType.Identity
    mult, add = mybir.AluOpType.mult, mybir.AluOpType.add

    nc = bacc.Bacc("TRN2", target_bir_lowering=False, debug=False)
    # x planes per sample: [CIN, 3(hi, lo, hi<<66), 4356] e4m3
    xT = nc.dram_tensor("xT", (_BPC, _CIN, 3, _FLAT), f8, kind="ExternalInput")
    # kernel tiles per (s, fh): [CIN, 12, 2, FH] e4m3 (see host prep)
    kT = nc.dram_tensor("kT", (_BPC, _NFH, _CIN, _NMM, 2, _FH), f8,
                        kind="ExternalInput")
    bT = nc.dram_tensor("bT", (_FH, _BPC * _NFH), f32, kind="ExternalInput")
    yT = nc.dram_tensor("yT", (_BPC, _NFH, _FH, _NOUT), f16,
                        kind="ExternalOutput")

    with tile.TileContext(nc) as tc:
        with (
            tc.tile_pool(name="xp", bufs=2) as xp,
            tc.tile_pool(name="kp", bufs=4) as kp,
            tc.tile_pool(name="bp", bufs=1) as bp,
            tc.tile_pool(name="op", bufs=4) as op,
            tc.tile_pool(name="osp", bufs=2) as osp,
            tc.tile_pool(name="ps", bufs=7, space="PSUM") as ps,
            tc.tile_pool(name="pss", bufs=1, space="PSUM") as pss,
        ):
            b_sb = None
            gate_prev = None
            for s in range(_BPC):
                dmas = []
                k_sb = []
                x_sb = xp.tile([_CIN, 3, _FLAT], f8, name=f"x{s}", tag="x")

                def load_k(fh, lo, hi, eng=None, s=s, k_sb=k_sb, dmas=dmas):
                    if lo == 0:
                        t = kp.tile([_CIN, _NMM, 2, _FH], f8,
                                    name=f"k{s}f{fh}", tag="k")
                        k_sb.append(t)
                    else:
                        t = k_sb[fh]
                    dmas.append((eng or nc.sync).dma_start(
                        t[:, lo:hi], kT[s, fh, :, lo:hi]))

                def load_x(a, bnd, eng=None, s=s, x_sb=x_sb, dmas=dmas):
                    dmas.append((eng or nc.sync).dma_start(
                        x_sb[:, :, a:bnd], xT[s, :, :, a:bnd]))

                if s == 0:
                    # critical prefix spread over SP/ACT/DVE issue queues so
                    # the 565ns/issue SP serialization stays off the critical
                    # path: k slots [0:2], x sliver, k rest, next x pieces
                    load_x(0, _XSPLITS[1], nc.sync)
                    load_k(0, 0, 2, nc.scalar)
                    load_k(0, 2, 7, nc.sync)
                    load_k(0, 7, _NMM, nc.scalar)
                    load_x(_XSPLITS[1], _XSPLITS[2], nc.sync)
                    b_sb = bp.tile([_FH, _BPC * _NFH], f32)
                    dmas.append(nc.scalar.dma_start(b_sb[:], bT[:]))
                    for piece in range(2, len(_XSPLITS) - 1):
                        load_x(_XSPLITS[piece], _XSPLITS[piece + 1])
                    load_k(1, 0, _NMM)
                else:
                    # prefetch: coarse pieces, SP queue
                    load_k(0, 0, _NMM)
                    load_x(0, 2178)
                    load_x(2178, _FLAT)
                    load_k(1, 0, _NMM)

                if gate_prev is not None:
                    # prefetch of sample s must not compete for HBM bandwidth
                    # with sample s-1's (still critical) input transfers
                    for d in dmas:
                        add_dep_helper(d.ins, gate_prev,
                                       reason="prefetch gated on prev sample")
                else:
                    # sample 0: keep late pieces off the wire until compute
                    # has started so the critical prefix gets full bandwidth
                    late = dmas[4:]

                gate_this = None
                for fh in range(_NFH):
                    col = s * _NFH + fh
                    bias_ap = b_sb[:, col:col + 1]
                    o_pair = None
                    # small chunk first (tiny x prefix -> early PE start),
                    # except the very last phase: small chunk last so the
                    # final evict+DMA tail is as short as possible
                    last_phase = s == _BPC - 1 and fh == _NFH - 1
                    chunks = (_CHUNKS[1:] + _CHUNKS[:1]) if last_phase \
                        else _CHUNKS
                    nbig = 0
                    for ci, (base, n) in enumerate(chunks):
                        small = n != 512
                        pool = pss if small else ps
                        psum = pool.tile([_FH, n], f32,
                                         name=f"ps_s{s}f{fh}c{ci}",
                                         tag="pss" if small else "psum")
                        for j in range(_NTAP):
                            o = base + _OFFS[j]
                            mm = nc.tensor.matmul(
                                psum[:], k_sb[fh][:, j],
                                x_sb[:, 0:2, o:o + n],
                                start=(j == 0), stop=False, perf_mode=DR,
                            )
                            if (gate_prev is None and s == 0 and fh == 0
                                    and ci == 0 and j == 0):
                                for d in late:
                                    add_dep_helper(
                                        d.ins, mm.ins,
                                        reason="s0 late inputs after first MM")
                            if fh == 0 and ci == 3 and j == 0:
                                gate_this = mm.ins
                        for p in range(_NPAIR):
                            # slots (x_hi[base+p], x_hi[base+p+66]) via the
                            # shifted plane: planes 0 and 2, step 2
                            nc.tensor.matmul(
                                psum[:], k_sb[fh][:, _NTAP + p],
                                x_sb[:, 0:3:2, base + p:base + p + n],
                                start=False, stop=(p == _NPAIR - 1),
                                perf_mode=DR,
                            )
                        # eviction: psum/64 + bias, alternating engines;
                        # last two chunks evict on separate engines with
                        # separate DMAs so the kernel tail drains fast
                        if small:
                            o_sb = osp.tile([_FH, n], f16,
                                            name=f"os_s{s}f{fh}", tag="os")
                            if last_phase:
                                # final tail: evict on DVE and issue the DMA
                                # from the DVE queue (no cross-engine wait,
                                # ACT still busy with the previous chunk)
                                nc.vector.tensor_scalar(
                                    o_sb[:], psum[:],
                                    1.0 / _KSCALE, bias_ap, mult, add)
                                nc.sync.dma_start(
                                    yT[s, fh, :, base:base + n], o_sb[:])
                            else:
                                nc.scalar.activation(o_sb[:], psum[:], ident,
                                                     bias=bias_ap,
                                                     scale=1.0 / _KSCALE)
                                nc.sync.dma_start(
                                    yT[s, fh, :, base:base + n], o_sb[:])
                            continue
                        bi = nbig
                        nbig += 1
                        on_dve = bi % 2 == 0
                        if bi >= 6:            # c7, c8: own tile + DMA
                            o_sb = osp.tile([_FH, 512], f16,
                                            name=f"ot_s{s}f{fh}b{bi}",
                                            tag=f"ot{bi % 2}")
                            dst, src = o_sb[:], psum[:]
                        elif on_dve:           # first of a pair
                            o_pair = op.tile([_FH, 1024], f16,
                                             name=f"o_s{s}f{fh}p{bi//2}",
                                             tag="o")
                            dst, src = o_pair[:, 0:512], psum[:]
                        else:                  # second of a pair
                            dst, src = o_pair[:, 512:1024], psum[:]
                        if on_dve:
                            nc.vector.tensor_scalar(
                                dst, src, 1.0 / _KSCALE, bias_ap, mult, add)
                        else:
                            nc.scalar.activation(
                                dst, src, ident,
                                bias=bias_ap, scale=1.0 / _KSCALE)
                        if bi >= 6:
                            # DVE-evicted chunk -> SP queue, ACT-evicted
                            # chunk -> ACT queue (no cross-engine wait)
                            eng = nc.sync if on_dve else nc.scalar
                            eng.dma_start(
                                yT[s, fh, :, base:base + 512], o_sb[:])
                        elif not on_dve:
                            nc.sync.dma_start(
                                yT[s, fh, :, base - 512:base + 512],
                                o_pair[:])
                gate_prev = gate_this
    nc.compile()
    return nc


def get_nc():
    global _nc
    if _nc is None:
        _nc = _build_nc()
    return _nc


def _prep_inputs(x, classes, kernel, bias):
    E = _e4m3()
    cls = np.asarray(classes)[:, 0]
    k_per = np.asarray(kernel)[cls]          # [B, KH, KW, CIN, F]
    b_per = np.asarray(bias)[cls]            # [B, F]

    # x -> padded flat planes, e4m3 hi/lo split + shifted-hi plane
    xpad = np.zeros((_B, _HP, _WP, _CIN), np.float32)
    xpad[:, 1:_H + 1, 1:_W + 1, :] = np.asarray(x, np.float32)
    xflat = np.ascontiguousarray(
        xpad.transpose(0, 3, 1, 2)).reshape(_B, _CIN, _FLAT)
    x_hi = xflat.astype(E)
    x_lo = (xflat - x_hi.astype(np.float32)).astype(E)
    x_sh = np.zeros_like(x_hi)
    x_sh[:, :, :_FLAT - _WP] = x_hi[:, :, _WP:]
    xT_all = np.stack([x_hi, x_lo, x_sh], axis=2)  # [B, CIN, 3, FLAT]

    # kernel -> 64x pre-scaled e4m3 hi/lo, packed into matmul slot tiles
    k64 = k_per.reshape(_B, _NTAP, _CIN, _NFH, _FH).astype(np.float32) * _KSCALE
    k_hi = k64.astype(E)
    k_lo = (k64 - k_hi.astype(np.float32)).astype(E)
    # [B, NFH, CIN, NTAP, FH]
    kA = np.ascontiguousarray(k_hi.transpose(0, 3, 2, 1, 4))
    kL = np.ascontiguousarray(k_lo.transpose(0, 3, 2, 1, 4))
    kT_all = np.zeros((_B, _NFH, _CIN, _NMM, 2, _FH), E)
    kT_all[:, :, :, :_NTAP, 0] = kA
    kT_all[:, :, :, :_NTAP, 1] = kA
    for p in range(_NPAIR):
        kT_all[:, :, :, _NTAP + p, 0] = kL[:, :, :, p]
        kT_all[:, :, :, _NTAP + p, 1] = kL[:, :, :, p + _KW]

    in_maps = []
    for i in range(_NCORES):
        lo = i * _BPC
        b_core = np.ascontiguousarray(
            b_per[lo:lo + _BPC].reshape(_BPC, _NFH, _FH)
            .astype(np.float32).transpose(2, 0, 1)
        ).reshape(_FH, _BPC * _NFH)
        in_maps.append({
            "xT": np.ascontiguousarray(xT_all[lo:lo + _BPC]),
            "kT": np.ascontiguousarray(kT_all[lo:lo + _BPC]),
            "bT": b_core,
        })
    return in_maps


# flat index of true output (r, c): p = r*66 + c
_GRID_IDX = (np.arange(_H)[:, None] * _WP + np.arange(_W)[None, :]).ravel()


def _unshard_one(yT):
    # yT: [BPC, NFH, FH, NOUT] f16 -> [BPC, H, W, F] f32
    y = yT.astype(np.float32)[:, :, :, _GRID_IDX]       # [BPC, 2, 128, H*W]
    y = y.reshape(_BPC, _F, _H * _W).transpose(0, 2, 1)  # [BPC, HW, F]
    return y.reshape(_BPC, _H, _W, _F)


def _unshard(results):
    outs = [_unshard_one(r["yT"]) for r in results]
    return np.ascontiguousarray(np.concatenate(outs, axis=0))


def run(x, classes, kernel, bias, trace=False):
    """Returns (y, BassKernelResults)."""
    from concourse.bass_utils import run_bass_kernel_spmd

    nc = get_nc()
    in_maps = _prep_inputs(x, classes, kernel, bias)
    res = run_bass_kernel_spmd(nc, in_maps, core_ids=list(range(_NCORES)),
                               trace=trace)
    return _unshard(res.results), res


def kernel(x, classes, kernel, bias):
    y, _ = run(x, classes, kernel, bias)
    return y


# revision 22
# speedup vs baseline: 1.0358x; 1.0358x over previous
"""Conditional (class-routed) 3x3 SAME conv, data-parallel over batch on 8 TRN2 cores.

Strategy (v2 - fp8 DoubleRow residual scheme):
  - Flat conv: zero-padded x [CIN, 66, 66] viewed as a flat [CIN, 4356] plane
    makes the 2D conv a 1D conv in the flat index (the 2 pad columns between
    rows absorb row wrap), so every tap window is a contiguous slice.  We
    compute 4222 flat outputs per (sample, f-half) (64 rows x 66, garbage at
    the 2 pad columns per row, stripped on host).
  - fp8e4 (e4m3) matmuls in MatmulPerfMode.DoubleRow: lhsT [K,2,M] x
    rhs [K,2,N] -> out[M,N] contracts a PAIR of products per output at half
    the per-row cost of fp16 -> 4x cheaper per product-term.
  - Residual precision scheme (kernel pre-scaled by 64, split k = k_hi+k_lo
    and x = x_hi+x_lo in e4m3):
      MM_A(tap t in 0..8): slots (x_hi*k_hi[t], x_lo*k_hi[t]) - exact-x
      MM_C(p in 0..2):     slots (x_hi*k_lo[t=p], x_hi*k_lo[t=p+3]) -
        k_lo corrections for the kh=0 and kh=1 tap rows, packed 2 per matmul
        using an extra "x_hi shifted by 66" SBUF plane (window offsets of the
        paired taps differ by exactly one padded row).
    Dropped terms: x*k_lo on the kh=2 tap row (dominant error, ~1.6e-2 rel)
    and x_lo*k_lo (~7e-4).  12 DoubleRow matmuls per chunk vs 9 fp16
    matmuls: 0.78x the fp16 PE-roofline time.
  - PSUM eviction applies out = psum/64 + bias, alternating DVE/ACT engines.
  - Host: gather per-sample expert kernels, build hi/lo e4m3 splits + the
    shifted plane, strip flat-index garbage and concatenate shards.
"""

import numpy as np

_B, _H, _W, _CIN = 32, 64, 64, 128
_F, _KH, _KW = 256, 3, 3
_NCORES = 8
_BPC = _B // _NCORES          # 4 samples per core
_HP, _WP = _H + 2, _W + 2     # 66, 66 (zero-padded)
_FLAT = _HP * _WP             # 4356 flat padded positions
_NOUT = (_H - 1) * _WP + _W   # 4222 flat outputs (incl. per-row garbage cols)
_FH = 128                     # output-channel half (PSUM partition dim)
_NFH = _F // _FH              # 2
_NTAP = _KH * _KW             # 9
_KSCALE = 64.0                # kernel pre-scale (keeps e4m3 out of subnormals)

_OFFS = [kh * _WP + kw for kh in range(_KH) for kw in range(_KW)]
_NPAIR = 3                    # MM_C count: pair p corrects taps (p, p+3)
_NMM = _NTAP + _NPAIR         # 12 matmuls per chunk
# small chunk FIRST: it needs only a sliver of x, so the PE starts early and
# the expensive 512-chunks run fully pstate-ramped
_CHUNKS = [(0, _NOUT - 8 * 512)] + [(126 + i * 512, 512) for i in range(8)]
_XSPLITS = (0, 512, 1060, 1810, 2560, 3560, _FLAT)  # x DMA piece boundaries

_nc = None
_E4M3 = None


def _e4m3():
    global _E4M3
    if _E4M3 is None:
        import concourse.mybir as mybir
        _E4M3 = np.dtype(mybir.dt.np(mybir.dt.float8e4))
    return _E4M3


def _build_nc():
    import concourse.bacc as bacc
    import concourse.mybir as mybir
    import concourse.tile as tile
    from concourse.tile_rust import add_dep_helper

    f32 = mybir.dt.float32
    f16 = mybir.dt.float16
    f8 = mybir.dt.float8e4
    DR = mybir.MatmulPerfMode.DoubleRow
    ident = mybir.ActivationFunctionType.Identity
    mult, add = mybir.AluOpType.mult, mybir.AluOpType.add

    nc = bacc.Bacc("TRN2", target_bir_lowering=False, debug=False)
    # x planes per sample: [CIN, 3(hi, lo, hi<<66), 4356] e4m3
    xT = nc.dram_tensor("xT", (_BPC, _CIN, 3, _FLAT), f8, kind="ExternalInput")
    # kernel tiles per (s, fh): [CIN, 12, 2, FH] e4m3 (see host prep)
    kT = nc.dram_tensor("kT", (_BPC, _NFH, _CIN, _NMM, 2, _FH), f8,
                        kind="ExternalInput")
    bT = nc.dram_tensor("bT", (_FH, _BPC * _NFH), f32, kind="ExternalInput")
    yT = nc.dram_tensor("yT", (_BPC, _NFH, _FH, _NOUT), f16,
                        kind="ExternalOutput")

    with tile.TileContext(nc) as tc:
        with (
            tc.tile_pool(name="xp", bufs=2) as xp,
            tc.tile_pool(name="kp", bufs=4) as kp,
            tc.tile_pool(name="bp", bufs=1) as bp,
            tc.tile_pool(name="op", bufs=4) as op,
            tc.tile_pool(name="osp", bufs=2) as osp,
            tc.tile_pool(name="ps", bufs=7, space="PSUM") as ps,
            tc.tile_pool(name="pss", bufs=1, space="PSUM") as pss,
        ):
            b_sb = None
            gate_prev = None
            for s in range(_BPC):
                dmas = []
                k_sb = []
                x_sb = xp.tile([_CIN, 3, _FLAT], f8, name=f"x{s}", tag="x")

                def load_k(fh, lo, hi, eng=None, s=s, k_sb=k_sb, dmas=dmas):
                    if lo == 0:
                        t = kp.tile([_CIN, _NMM, 2, _FH], f8,
                                    name=f"k{s}f{fh}", tag="k")
                        k_sb.append(t)
                    else:
                        t = k_sb[fh]
                    dmas.append((eng or nc.sync).dma_start(
                        t[:, lo:hi], kT[s, fh, :, lo:hi]))

                def load_x(a, bnd, eng=None, s=s, x_sb=x_sb, dmas=dmas):
                    dmas.append((eng or nc.sync).dma_start(
                        x_sb[:, :, a:bnd], xT[s, :, :, a:bnd]))

                if s == 0:
                    # SP queue only (transfers serialize on the DMA device
                    # anyway; the ACT queue is blocked early by the
                    # LoadActFuncSet).  Order = need order; everything past
                    # xB/bias is gated behind the first matmul.
                    load_x(0, _XSPLITS[1])
                    load_k(0, 0, 2)
                    load_k(0, 2, 7)
                    load_k(0, 7, _NMM)
                    load_x(_XSPLITS[1], _XSPLITS[2])
                    b_sb = bp.tile([_FH, _BPC * _NFH], f32)
                    dmas.append(nc.sync.dma_start(b_sb[:], bT[:]))
                    n_crit = len(dmas)
                    for piece in range(2, len(_XSPLITS) - 1):
                        load_x(_XSPLITS[piece], _XSPLITS[piece + 1])
                    load_k(1, 0, _NMM)
                else:
                    # prefetch: k tiles via the ACT queue to offload SP
                    load_k(0, 0, _NMM, nc.scalar)
                    load_x(0, 2178)
                    load_x(2178, _FLAT)
                    load_k(1, 0, _NMM, nc.scalar)

                if gate_prev is not None:
                    # prefetch of sample s must not compete for HBM bandwidth
                    # with sample s-1's (still critical) input transfers
                    for d in dmas:
                        add_dep_helper(d.ins, gate_prev,
                                       reason="prefetch gated on prev sample")
                else:
                    # sample 0: keep late pieces off the wire until compute
                    # has started so the critical prefix gets full bandwidth
                    late = dmas[n_crit:]

                gate_this = None
                for fh in range(_NFH):
                    col = s * _NFH + fh
                    bias_ap = b_sb[:, col:col + 1]
                    o_pair = None
                    # small chunk first (tiny x prefix -> early PE start),
                    # except the very last phase: small chunk last so the
                    # final evict+DMA tail is as short as possible
                    last_phase = s == _BPC - 1 and fh == _NFH - 1
                    chunks = (_CHUNKS[1:] + _CHUNKS[:1]) if last_phase \
                        else _CHUNKS
                    nbig = 0
                    for ci, (base, n) in enumerate(chunks):
                        small = n != 512
                        pool = pss if small else ps
                        psum = pool.tile([_FH, n], f32,
                                         name=f"ps_s{s}f{fh}c{ci}",
                                         tag="pss" if small else "psum")
                        for j in range(_NTAP):
                            o = base + _OFFS[j]
                            mm = nc.tensor.matmul(
                                psum[:], k_sb[fh][:, j],
                                x_sb[:, 0:2, o:o + n],
                                start=(j == 0), stop=False, perf_mode=DR,
                            )
                            if (gate_prev is None and s == 0 and fh == 0
                                    and ci == 0 and j == 0):
                                for d in late:
                                    add_dep_helper(
                                        d.ins, mm.ins,
                                        reason="s0 late inputs after first MM")
                            if fh == 0 and ci == 3 and j == 0:
                                gate_this = mm.ins
                        for p in range(_NPAIR):
                            # slots (x_hi[base+p], x_hi[base+p+66]) via the
                            # shifted plane: planes 0 and 2, step 2
                            nc.tensor.matmul(
                                psum[:], k_sb[fh][:, _NTAP + p],
                                x_sb[:, 0:3:2, base + p:base + p + n],
                                start=False, stop=(p == _NPAIR - 1),
                                perf_mode=DR,
                            )
                        # eviction: psum/64 + bias, alternating engines;
                        # last two chunks evict on separate engines with
                        # separate DMAs so the kernel tail drains fast
                        if small:
                            o_sb = osp.tile([_FH, n], f16,
                                            name=f"os_s{s}f{fh}", tag="os")
                            if last_phase:
                                # final tail: evict on DVE and issue the DMA
                                # from the DVE queue (no cross-engine wait,
                                # ACT still busy with the previous chunk)
                                nc.vector.tensor_scalar(
                                    o_sb[:], psum[:],
                                    1.0 / _KSCALE, bias_ap, mult, add)
                                nc.sync.dma_start(
                                    yT[s, fh, :, base:base + n], o_sb[:])
                            else:
                                nc.scalar.activation(o_sb[:], psum[:], ident,
                                                     bias=bias_ap,
                                                     scale=1.0 / _KSCALE)
                                nc.sync.dma_start(
                                    yT[s, fh, :, base:base + n], o_sb[:])
                            continue
                        bi = nbig
                        nbig += 1
                        on_dve = bi % 2 == 0
                        if bi >= 6:            # c7, c8: own tile + DMA
                            o_sb = osp.tile([_FH, 512], f16,
                                            name=f"ot_s{s}f{fh}b{bi}",
                                            tag=f"ot{bi % 2}")
                            dst, src = o_sb[:], psum[:]
                        elif on_dve:           # first of a pair
                            o_pair = op.tile([_FH, 1024], f16,
                                             name=f"o_s{s}f{fh}p{bi//2}",
                                             tag="o")
                            dst, src = o_pair[:, 0:512], psum[:]
                        else:                  # second of a pair
                            dst, src = o_pair[:, 512:1024], psum[:]
                        if on_dve:
                            nc.vector.tensor_scalar(
                                dst, src, 1.0 / _KSCALE, bias_ap, mult, add)
                        else:
                            nc.scalar.activation(
                                dst, src, ident,
                                bias=bias_ap, scale=1.0 / _KSCALE)
                        if bi >= 6:
                            # DVE-evicted chunk -> SP queue, ACT-evicted
                            # chunk -> ACT queue (no cross-engine wait)
                            eng = nc.sync if on_dve else nc.scalar
                            eng.dma_start(
                                yT[s, fh, :, base:base + 512], o_sb[:])
                        elif not on_dve:
                            nc.sync.dma_start(
                                yT[s, fh, :, base - 512:base + 512],
                                o_pair[:])
                gate_prev = gate_this
    nc.compile()
    return nc


def get_nc():
    global _nc
    if _nc is None:
        _nc = _build_nc()
    return _nc


def _prep_inputs(x, classes, kernel, bias):
    E = _e4m3()
    cls = np.asarray(classes)[:, 0]
    k_per = np.asarray(kernel)[cls]          # [B, KH, KW, CIN, F]
    b_per = np.asarray(bias)[cls]            # [B, F]

    # x -> padded flat planes, e4m3 hi/lo split + shifted-hi plane
    xpad = np.zeros((_B, _HP, _WP, _CIN), np.float32)
    xpad[:, 1:_H + 1, 1:_W + 1, :] = np.asarray(x, np.float32)
    xflat = np.ascontiguousarray(
        xpad.transpose(0, 3, 1, 2)).reshape(_B, _CIN, _FLAT)
    x_hi = xflat.astype(E)
    x_lo = (xflat - x_hi.astype(np.float32)).astype(E)
    x_sh = np.zeros_like(x_hi)
    x_sh[:, :, :_FLAT - _WP] = x_hi[:, :, _WP:]
    xT_all = np.stack([x_hi, x_lo, x_sh], axis=2)  # [B, CIN, 3, FLAT]

    # kernel -> 64x pre-scaled e4m3 hi/lo, packed into matmul slot tiles
    k64 = k_per.reshape(_B, _NTAP, _CIN, _NFH, _FH).astype(np.float32) * _KSCALE
    k_hi = k64.astype(E)
    k_lo = (k64 - k_hi.astype(np.float32)).astype(E)
    # [B, NFH, CIN, NTAP, FH]
    kA = np.ascontiguousarray(k_hi.transpose(0, 3, 2, 1, 4))
    kL = np.ascontiguousarray(k_lo.transpose(0, 3, 2, 1, 4))
    kT_all = np.zeros((_B, _NFH, _CIN, _NMM, 2, _FH), E)
    kT_all[:, :, :, :_NTAP, 0] = kA
    kT_all[:, :, :, :_NTAP, 1] = kA
    for p in range(_NPAIR):
        kT_all[:, :, :, _NTAP + p, 0] = kL[:, :, :, p]
        kT_all[:, :, :, _NTAP + p, 1] = kL[:, :, :, p + _KW]

    in_maps = []
    for i in range(_NCORES):
        lo = i * _BPC
        b_core = np.ascontiguousarray(
            b_per[lo:lo + _BPC].reshape(_BPC, _NFH, _FH)
            .astype(np.float32).transpose(2, 0, 1)
        ).reshape(_FH, _BPC * _NFH)
        in_maps.append({
            "xT": np.ascontiguousarray(xT_all[lo:lo + _BPC]),
            "kT": np.ascontiguousarray(kT_all[lo:lo + _BPC]),
            "bT": b_core,
        })
    return in_maps


# flat index of true output (r, c): p = r*66 + c
_GRID_IDX = (np.arange(_H)[:, None] * _WP + np.arange(_W)[None, :]).ravel()


def _unshard_one(yT):
    # yT: [BPC, NFH, FH, NOUT] f16 -> [BPC, H, W, F] f32
    y = yT.astype(np.float32)[:, :, :, _GRID_IDX]       # [BPC, 2, 128, H*W]
    y = y.reshape(_BPC, _F, _H * _W).transpose(0, 2, 1)  # [BPC, HW, F]
    return y.reshape(_BPC, _H, _W, _F)


def _unshard(results):
    outs = [_unshard_one(r["yT"]) for r in results]
    return np.ascontiguousarray(np.concatenate(outs, axis=0))


def run(x, classes, kernel, bias, trace=False):
    """Returns (y, BassKernelResults)."""
    from concourse.bass_utils import run_bass_kernel_spmd

    nc = get_nc()
    in_maps = _prep_inputs(x, classes, kernel, bias)
    res = run_bass_kernel_spmd(nc, in_maps, core_ids=list(range(_NCORES)),
                               trace=trace)
    return _unshard(res.results), res


def kernel(x, classes, kernel, bias):
    y, _ = run(x, classes, kernel, bias)
    return y
